# revision 13
# baseline (speedup 1.0000x reference)
"""Causal self-attention (separate heads) TRN2 Bass kernel — v3.

Problem (hardcoded): B=4, T=2048, C=1024, H=16, HS=64, fp32 in/out.
  q/k/v = per-head linear projections of x; att = softmax(causal(q k^T / 8));
  y = att v; out = concat_heads(y) @ Wp.T + bp.

Sharding over 8 NeuronCores: core c -> batch b = c//2, head-group hg = c%2
(8 heads each). Each core computes a [T, C] partial of the output; the host
sums the two partials per batch and adds bp.

All matmul operands bf16 (PSUM accumulation fp32); out DMA'd bf16 and summed
in fp32 on the host. x and all weights are DMA'd once and SBUF-resident.

Schedule (single PE stream, block-granular ordering):
  warm-up dummy MMs (HAM clock release) + bvrep broadcast
  QK0 t01  ->  V st0-7  (K=1024 single accumulation groups)
  stage p=0..3: per (j, i): two S MMs (head-halves h0/h64 row groups) into a
    2-bank psS2 -> ONE wide exp (ACT overhead is 352 cycles/instr) -> causal
    tri-mask mul -> AV with ones-augmented V -> chunk tails copy psY to SBUF
    (frees PSUM), reciprocal_approx_fast (SBUF only - PSUM reads broken),
    gpsimd broadcast, normalize mul into YT.
    Fillers: s0: QK0 t23 + V st8-15 + QK1; s1: QK1? -> see stage_fill; C units
    (c_proj per m-block, both e-halves, one DMA) interleave into stage 3 as
    its tails complete.
"""
import numpy as np
import ml_dtypes

from concourse import bacc, bass_utils, tile, mybir

B, T, C, H, HS = 4, 2048, 1024, 16, 64
NCORE = 8
NPAIR = 4
NCH = T // 512
NST = T // 128

f32 = mybir.dt.float32
bf16 = mybir.dt.bfloat16
EXP = mybir.ActivationFunctionType.Exp

_CACHE = {}


def _build():
    nc = bacc.Bacc(None, target_bir_lowering=False)

    xall_d = nc.declare_dram_parameter("xall", [128, 8, T], bf16, isOutput=False)
    wq_d = nc.declare_dram_parameter("wq", [128, 8, 512], bf16, isOutput=False)
    wk_d = nc.declare_dram_parameter("wk", [128, 8, 512], bf16, isOutput=False)
    wv_d = nc.declare_dram_parameter("wv", [128, 8, 512], bf16, isOutput=False)
    wp_d = nc.declare_dram_parameter("wp", [128, 4, 1024], bf16, isOutput=False)
    bq_d = nc.declare_dram_parameter("bq", [128, 4], f32, isOutput=False)
    bk_d = nc.declare_dram_parameter("bk", [128, 4], f32, isOutput=False)
    bv_d = nc.declare_dram_parameter("bv", [1, 512], bf16, isOutput=False)
    tri_d = nc.declare_dram_parameter("trimask", [128, 128], bf16, isOutput=False)
    out = nc.declare_dram_parameter("out", [T, C], bf16, isOutput=True)

    with tile.TileContext(nc) as tc:
        with tc.tile_pool(name="persist", bufs=1) as pp:
            # ---- constants / persistent tensors ----
            ones_sb = pp.tile([1, 128], bf16, tag="ones")
            warm_rhs = pp.tile([1, 512], bf16, tag="warmrow")
            onescol = pp.tile([128, 32], f32, tag="onescol")
            nc.vector.memset(ones_sb[:], 1.0)
            nc.vector.memset(warm_rhs[:], 1.0)
            nc.vector.memset(onescol[:], 1.0)

            tri_sb = pp.tile([128, 128], bf16, tag="tri")
            bq_sb = pp.tile([128, 4], f32, tag="bq")
            bk_sb = pp.tile([128, 4], f32, tag="bk")
            bv_sb = pp.tile([1, 512], bf16, tag="bv")
            xall = pp.tile([128, 8, T], bf16, tag="xall")
            wv_sb = pp.tile([128, 8, 512], bf16, tag="wv")
            wq_sb = pp.tile([128, 8, 512], bf16, tag="wq")
            wk_sb = pp.tile([128, 8, 512], bf16, tag="wk")
            wp_sb = pp.tile([128, 4, 1024], bf16, tag="wp")

            # DMA priority order: tiny consts, then what the preamble needs
            # first (xall + wq/wk for QK0, wv for V), then wp.
            nc.sync.dma_start(bv_sb[:], bv_d[:])
            nc.sync.dma_start(bq_sb[:], bq_d[:])
            nc.sync.dma_start(bk_sb[:], bk_d[:])
            nc.sync.dma_start(tri_sb[:], tri_d[:])
            for k in range(8):
                nc.sync.dma_start(xall[:, k, :], xall_d[:, k, :])
            nc.sync.dma_start(wq_sb[:], wq_d[:])
            nc.sync.dma_start(wk_sb[:], wk_d[:])
            nc.sync.dma_start(wv_sb[:], wv_d[:])
            nc.sync.dma_start(wp_sb[:], wp_d[:])

            # per-(pair, i, hh) block padded to 72 cols (144B) so every AV
            # lhsT slice is 16B-aligned (bf16 weight loads corrupt otherwise)
            V = pp.tile([128, NPAIR, NST, 2, 72], bf16, tag="V")
            for p in range(NPAIR):
                for i in range(NST):
                    nc.vector.tensor_copy(V[:, p, i, :, 64], onescol[:, 0:2])

            # PE emission-order chain, block granular
            _chain = {"prev": None, "first": None}

            def pe_mm(*args, **kw):
                inst = nc.tensor.matmul(*args, **kw)
                if _chain["first"] is None and _chain["prev"] is not None:
                    tile.add_dep_helper(
                        inst.ins, _chain["prev"].ins, sync=False,
                        reason="pe block order",
                    )
                if _chain["first"] is None:
                    _chain["first"] = inst
                _chain["prev"] = inst
                return inst

            def end_blk():
                _chain["first"] = None

            bvrep = pp.tile([128, 512], f32, tag="bvrep")
            with tc.tile_pool(name="ps_init", bufs=2, space="PSUM") as ps_init:
                # HAM warm-up: ~4.3us of dummy matmuls (no DMA deps) so the
                # PE clock gate releases (1.2 -> 2.4 GHz) before real work.
                trash = ps_init.tile([128, 512], f32, tag="trash", name="trash")
                NWARM = 16
                for dnum in range(NWARM):
                    pe_mm(
                        trash[:], ones_sb[:], warm_rhs[:],
                        start=(dnum == 0), stop=(dnum == NWARM - 1),
                    )
                end_blk()
                for p in range(NPAIR):
                    psb = ps_init.tile([128, 512], f32, tag="psb", name="psb")
                    pe_mm(
                        psb[:, 0:128], ones_sb[:], bv_sb[:, 128 * p : 128 * p + 128],
                        start=True, stop=True,
                    )
                    end_blk()
                    nc.vector.tensor_copy(
                        bvrep[:, 128 * p : 128 * p + 128], psb[:, 0:128]
                    )

            with tc.tile_pool(name="phBC", bufs=1) as pb:
              YT = pb.tile([128, NPAIR, T], bf16, tag="YT")
              with (
                  tc.tile_pool(name="qkt", bufs=2) as pqkt,
                  tc.tile_pool(name="phB_es", bufs=8) as pes,
                  tc.tile_pool(name="phB_rep", bufs=2) as prep,
                  tc.tile_pool(name="phC", bufs=3) as pc,
                  tc.tile_pool(name="ps_work", bufs=3, space="PSUM") as pwork,
                  tc.tile_pool(name="ps_Y", bufs=1, space="PSUM") as psy,
              ):
                qt_of = {}
                kt_of = {}

                def alloc_qkt(p):
                    qt_of[p] = pqkt.tile([128, T], bf16, tag="QTp", name="QTp")
                    kt_of[p] = pqkt.tile([128, T], bf16, tag="KTp", name="KTp")

                def work_tile():
                    return pwork.tile([128, 2, 512], f32, tag="pw", name="pw")

                def qk_unit(p, proj, tp):
                    """Q or K projection of pair p for t-chunk-pair tp
                    (columns 1024*tp .. 1024*tp+1024), K=1024 contraction in
                    one PSUM accumulation group per 512-wide half."""
                    def go():
                        if p not in qt_of:
                            alloc_qkt(p)
                        w_sb = wq_sb if proj == "q" else wk_sb
                        dest = qt_of[p] if proj == "q" else kt_of[p]
                        bias_sb = bq_sb if proj == "q" else bk_sb
                        ps = work_tile()
                        for half in range(2):
                            tch = 2 * tp + half
                            for kk in range(8):
                                pe_mm(
                                    ps[:, half, :],
                                    w_sb[:, kk, 128 * p : 128 * p + 128],
                                    xall[:, kk, 512 * tch : 512 * tch + 512],
                                    start=(kk == 0),
                                    stop=(kk == 7),
                                )
                        end_blk()
                        nc.vector.tensor_scalar_add(
                            dest[:, 1024 * tp : 1024 * tp + 1024],
                            ps.rearrange("s a b -> s (a b)"),
                            bias_sb[:, p : p + 1],
                        )
                    return go

                def v_unit(u):
                    """V for st-pair (2u, 2u+1), all pairs, K=1024 single
                    accumulation group per 512-wide half."""
                    def go():
                        ps = work_tile()
                        for half in range(2):
                            st = 2 * u + half
                            for kk in range(8):
                                pe_mm(
                                    ps[:, half, :],
                                    xall[:, kk, 128 * st : 128 * st + 128],
                                    wv_sb[:, kk, :],
                                    start=(kk == 0),
                                    stop=(kk == 7),
                                )
                        end_blk()
                        for half in range(2):
                            st = 2 * u + half
                            src4 = ps[:, half, :].rearrange(
                                "s (p two d) -> s p two d", p=4, two=2
                            )
                            b4 = bvrep.rearrange(
                                "s (p two d) -> s p two d", p=4, two=2
                            )
                            for hh in range(2):
                                nc.vector.tensor_add(
                                    V[:, :, st, hh, 0:64],
                                    src4[:, :, hh, :],
                                    b4[:, :, hh, :],
                                )
                    return go

                def c_unit(m):
                    """c_proj for t-block m: both 512-wide e-halves, summed
                    over pairs in PSUM, one bf16 copy + one DMA out."""
                    def go():
                        ps = work_tile()
                        for e in range(2):
                            for pp_ in range(NPAIR):
                                pe_mm(
                                    ps[:, e, :],
                                    YT[:, pp_, 128 * m : 128 * m + 128],
                                    wp_sb[:, pp_, 512 * e : 512 * e + 512],
                                    start=(pp_ == 0),
                                    stop=(pp_ == 3),
                                )
                        end_blk()
                        ob = pc.tile([128, 1024], bf16, tag="ob", name="ob")
                        nc.vector.tensor_copy(
                            ob[:], ps.rearrange("s a b -> s (a b)")
                        )
                        nc.sync.dma_start(
                            out[128 * m : 128 * m + 128, :], ob[:]
                        )
                    return go

                def emit_tail(p, j, psY):
                    """reciprocal_approx_fast only works from SBUF at base
                    partition 0 — stage the ones-row through a [1,512] tile."""
                    for hh in range(2):
                        row = prep.tile([1, 512], f32, tag="row", name="row")
                        nc.vector.tensor_copy(row[:], psY[hh][64:65, :])
                        rrow = prep.tile([1, 512], f32, tag="rrow", name="rrow")
                        nc.vector.reciprocal_approx_fast(
                            out=rrow[:], in_=row[:]
                        )
                        repc = prep.tile([64, 512], f32, tag="repc", name="repc")
                        nc.gpsimd.partition_broadcast(repc[:], rrow[:])
                        nc.vector.tensor_mul(
                            YT[64 * hh : 64 * hh + 64, p, 512 * j : 512 * j + 512],
                            psY[hh][0:64, :],
                            repc[:],
                        )

                # ---------------- preamble: QK0 t01, V st0-7 ----------------
                qk_unit(0, "q", 0)()
                qk_unit(0, "k", 0)()
                for u in range(4):
                    v_unit(u)()

                # ---------------- attention stages ----------------
                stage_fill = {
                    0: [qk_unit(0, "q", 1), qk_unit(0, "k", 1),
                        v_unit(4), v_unit(5), v_unit(6), v_unit(7),
                        qk_unit(1, "q", 0), qk_unit(1, "k", 0)],
                    1: [qk_unit(1, "q", 1), qk_unit(1, "k", 1),
                        qk_unit(2, "q", 0), qk_unit(2, "k", 0)],
                    2: [qk_unit(2, "q", 1), qk_unit(2, "k", 1),
                        qk_unit(3, "q", 0), qk_unit(3, "k", 0)],
                    3: [qk_unit(3, "q", 1), qk_unit(3, "k", 1)],
                }
                LOOK = 4  # S -> AV lookahead in (j, i) blocks

                for stage in range(NPAIR):
                    p = stage
                    filler = stage_fill[p]
                    fidx = 0
                    last_stage = stage == NPAIR - 1

                    blocks = []  # (j, i, last_of_chunk)
                    for j in range(NCH):
                        nst_j = 4 * j + 4
                        for i in range(nst_j):
                            blocks.append((j, i, i == nst_j - 1))
                    nblk = len(blocks)
                    nfe = len(filler) + (16 if last_stage else 0)

                    eS_store = {}
                    psY_of = {}
                    tails_pending = []

                    def pop_tails(n, filler=filler, last_stage=last_stage):
                        while tails_pending and tails_pending[0][0] <= n:
                            _, tp_, tj, tpsY = tails_pending.pop(0)
                            emit_tail(tp_, tj, tpsY)
                            if last_stage:
                                for m in range(4 * tj, 4 * tj + 4):
                                    filler.append(c_unit(m))

                    for n in range(nblk + LOOK):
                        pop_tails(n)
                        # AV block n-LOOK
                        if n >= LOOK:
                            j, i, last = blocks[n - LOOK]
                            psY = psY_of[j]
                            nst_j = 4 * j + 4
                            off = max(0, 128 * i - 512 * j)
                            eS = eS_store.pop((j, i))
                            for hh in range(2):
                                pe_mm(
                                    psY[hh][:, off:512],
                                    V[:, p, i, hh, 0:65],
                                    eS[:, hh, off:512],
                                    start=(i == 0),
                                    stop=(i == nst_j - 1),
                                )
                            end_blk()
                            if last:
                                tails_pending.append((n + 1, p, j, psY))
                        # filler unit(s), front-loaded, capped per block
                        want = min(
                            len(filler),
                            ((n + 1) * nfe) // max(1, int(0.6 * nblk)),
                            fidx + 2,
                        )
                        while fidx < want:
                            filler[fidx]()
                            fidx += 1
                        # S block n: both head-halves -> one wide exp
                        if n < nblk:
                            j, i, last = blocks[n]
                            if j not in psY_of:
                                psY_of[j] = [
                                    psy.tile(
                                        [65, 512], f32,
                                        tag=f"psY{hh}", name=f"psY{hh}",
                                    )
                                    for hh in range(2)
                                ]
                            off = max(0, 128 * i - 512 * j)
                            psS = work_tile()
                            for hh in range(2):
                                h0 = 64 * hh
                                pe_mm(
                                    psS[:, hh, off:512],
                                    kt_of[p][h0 : h0 + 64, 128 * i : 128 * i + 128],
                                    qt_of[p][
                                        h0 : h0 + 64,
                                        512 * j + off : 512 * j + 512,
                                    ],
                                    start=True,
                                    stop=True,
                                )
                            end_blk()
                            eS = pes.tile([128, 2, 512], bf16, tag="eS", name="eS")
                            nc.scalar.activation(
                                eS[:, :, off:512], psS[:, :, off:512], EXP,
                                scale=0.125,
                            )
                            if i >= 4 * j:
                                for hh in range(2):
                                    nc.vector.tensor_mul(
                                        eS[:, hh, off : off + 128],
                                        eS[:, hh, off : off + 128],
                                        tri_sb[:],
                                    )
                            eS_store[(j, i)] = eS
                    # drain: tails first (they append C units on stage 3)
                    pop_tails(10**9)
                    while fidx < len(filler):
                        filler[fidx]()
                        fidx += 1

    nc.compile()
    return nc


def _prep_core_inputs(x, Wq, bq, Wk, bk, Wv, bv, core):
    b, hg = core // 2, core % 2
    h0 = 8 * hg
    bft = ml_dtypes.bfloat16

    def wprep(W):
        A = W[h0 : h0 + 8]
        Bm = np.transpose(A, (2, 0, 1)).reshape(C, 512)
        return np.ascontiguousarray(
            Bm.reshape(8, 128, 512).transpose(1, 0, 2)
        ).astype(bft)

    def bprep(bias):
        return np.ascontiguousarray(bias[h0 : h0 + 8].reshape(4, 128).T)

    xT = x[b].T  # [C, T]
    xall = np.ascontiguousarray(
        xT.reshape(8, 128, T).transpose(1, 0, 2)
    ).astype(bft)

    return {
        "xall": xall,
        "wq": wprep(Wq),
        "wk": wprep(Wk),
        "wv": wprep(Wv),
        "bq": bprep(bq),
        "bk": bprep(bk),
        "bv": np.ascontiguousarray(bv[h0 : h0 + 8].reshape(1, 512)).astype(bft),
        "trimask": np.triu(np.ones((128, 128), np.float32)).astype(bft),
    }


def _prep_wp(Wp, hg):
    wp_sl = Wp[:, 512 * hg : 512 * hg + 512]
    return np.ascontiguousarray(
        wp_sl.T.reshape(4, 128, 1024).transpose(1, 0, 2)
    ).astype(ml_dtypes.bfloat16)


TRACE = False
TRACE_KW = {}


def kernel(x, Wq, bq, Wk, bk, Wv, bv, Wp, bp):
    x = np.asarray(x, np.float32)
    Wq = np.asarray(Wq, np.float32)
    bq = np.asarray(bq, np.float32)
    Wk = np.asarray(Wk, np.float32)
    bk = np.asarray(bk, np.float32)
    Wv = np.asarray(Wv, np.float32)
    bv = np.asarray(bv, np.float32)
    Wp = np.asarray(Wp, np.float32)
    bp = np.asarray(bp, np.float32)

    if "nc" not in _CACHE:
        _CACHE["nc"] = _build()
    nc = _CACHE["nc"]

    wp_of_hg = [_prep_wp(Wp, hg) for hg in range(2)]
    in_maps = []
    for core in range(NCORE):
        m = _prep_core_inputs(x, Wq, bq, Wk, bk, Wv, bv, core)
        m["wp"] = wp_of_hg[core % 2]
        in_maps.append(m)
    res = bass_utils.run_bass_kernel_spmd(
        nc, in_maps, list(range(NCORE)), trace=TRACE, **TRACE_KW
    )
    _CACHE["last_result"] = res

    outp = np.empty((B, T, C), np.float32)
    for b in range(B):
        outp[b] = (
            res.results[2 * b]["out"].astype(np.float32)
            + res.results[2 * b + 1]["out"].astype(np.float32)
            + bp
        )
    return outp


# revision 15
# speedup vs baseline: 1.0717x; 1.0717x over previous
"""Causal self-attention (separate heads) TRN2 Bass kernel — v3.

Problem (hardcoded): B=4, T=2048, C=1024, H=16, HS=64, fp32 in/out.
  q/k/v = per-head linear projections of x; att = softmax(causal(q k^T / 8));
  y = att v; out = concat_heads(y) @ Wp.T + bp.

Sharding over 8 NeuronCores: core c -> batch b = c//2, head-group hg = c%2
(8 heads each). Each core computes a [T, C] partial of the output; the host
sums the two partials per batch and adds bp.

All matmul operands bf16 (PSUM accumulation fp32); out DMA'd bf16 and summed
in fp32 on the host. x and all weights are DMA'd once and SBUF-resident.

Schedule (single PE stream, block-granular ordering):
  warm-up dummy MMs (HAM clock release) + bvrep broadcast
  QK0 t01  ->  V st0-7  (K=1024 single accumulation groups)
  stage p=0..3: per (j, i): two S MMs (head-halves h0/h64 row groups) into a
    2-bank psS2 -> ONE wide exp (ACT overhead is 352 cycles/instr) -> causal
    tri-mask mul -> AV with ones-augmented V -> chunk tails copy psY to SBUF
    (frees PSUM), reciprocal_approx_fast (SBUF only - PSUM reads broken),
    gpsimd broadcast, normalize mul into YT.
    Fillers: s0: QK0 t23 + V st8-15 + QK1; s1: QK1? -> see stage_fill; C units
    (c_proj per m-block, both e-halves, one DMA) interleave into stage 3 as
    its tails complete.
"""
import numpy as np
import ml_dtypes

from concourse import bacc, bass_utils, tile, mybir

B, T, C, H, HS = 4, 2048, 1024, 16, 64
NCORE = 8
NPAIR = 4
NCH = T // 512
NST = T // 128

f32 = mybir.dt.float32
bf16 = mybir.dt.bfloat16
EXP = mybir.ActivationFunctionType.Exp

_CACHE = {}


def _build():
    nc = bacc.Bacc(None, target_bir_lowering=False)

    xall_d = nc.declare_dram_parameter("xall", [128, 8, T], bf16, isOutput=False)
    wq_d = nc.declare_dram_parameter("wq", [128, 8, 512], bf16, isOutput=False)
    wk_d = nc.declare_dram_parameter("wk", [128, 8, 512], bf16, isOutput=False)
    wv_d = nc.declare_dram_parameter("wv", [128, 8, 512], bf16, isOutput=False)
    wp_d = nc.declare_dram_parameter("wp", [128, 4, 1024], bf16, isOutput=False)
    bq_d = nc.declare_dram_parameter("bq", [128, 4], f32, isOutput=False)
    bk_d = nc.declare_dram_parameter("bk", [128, 4], f32, isOutput=False)
    bv_d = nc.declare_dram_parameter("bv", [1, 512], bf16, isOutput=False)
    tri_d = nc.declare_dram_parameter("trimask", [128, 128], bf16, isOutput=False)
    out = nc.declare_dram_parameter("out", [T, C], bf16, isOutput=True)

    with tile.TileContext(nc) as tc:
        with tc.tile_pool(name="persist", bufs=1) as pp:
            # ---- constants / persistent tensors ----
            ones_sb = pp.tile([1, 128], bf16, tag="ones")
            warm_rhs = pp.tile([1, 512], bf16, tag="warmrow")
            onescol = pp.tile([128, 32], f32, tag="onescol")
            nc.vector.memset(ones_sb[:], 1.0)
            nc.vector.memset(warm_rhs[:], 1.0)
            nc.vector.memset(onescol[:], 1.0)

            tri_sb = pp.tile([128, 128], bf16, tag="tri")
            bq_sb = pp.tile([128, 4], f32, tag="bq")
            bk_sb = pp.tile([128, 4], f32, tag="bk")
            bv_sb = pp.tile([1, 512], bf16, tag="bv")
            xall = pp.tile([128, 8, T], bf16, tag="xall")
            wv_sb = pp.tile([128, 8, 512], bf16, tag="wv")
            wq_sb = pp.tile([128, 8, 512], bf16, tag="wq")
            wk_sb = pp.tile([128, 8, 512], bf16, tag="wk")
            wp_sb = pp.tile([128, 4, 1024], bf16, tag="wp")

            # DMA priority order: tiny consts, then what the preamble needs
            # first (xall + wq/wk for QK0, wv for V), then wp.
            nc.sync.dma_start(bv_sb[:], bv_d[:])
            nc.sync.dma_start(bq_sb[:], bq_d[:])
            nc.sync.dma_start(bk_sb[:], bk_d[:])
            nc.sync.dma_start(tri_sb[:], tri_d[:])
            nc.sync.dma_start(wq_sb[:], wq_d[:])
            nc.sync.dma_start(wk_sb[:], wk_d[:])
            for k in range(8):
                nc.sync.dma_start(xall[:, k, :], xall_d[:, k, :])
            nc.sync.dma_start(wv_sb[:], wv_d[:])
            nc.sync.dma_start(wp_sb[:], wp_d[:])

            # per-(pair, i, hh) block padded to 72 cols (144B) so every AV
            # lhsT slice is 16B-aligned (bf16 weight loads corrupt otherwise)
            V = pp.tile([128, NPAIR, NST, 2, 72], bf16, tag="V")
            for p in range(NPAIR):
                for i in range(NST):
                    nc.vector.tensor_copy(V[:, p, i, :, 64], onescol[:, 0:2])

            # PE emission-order chain, block granular
            _chain = {"prev": None, "first": None}

            def pe_mm(*args, **kw):
                inst = nc.tensor.matmul(*args, **kw)
                if _chain["first"] is None and _chain["prev"] is not None:
                    tile.add_dep_helper(
                        inst.ins, _chain["prev"].ins, sync=False,
                        reason="pe block order",
                    )
                if _chain["first"] is None:
                    _chain["first"] = inst
                _chain["prev"] = inst
                return inst

            def end_blk():
                _chain["first"] = None

            bvrep = pp.tile([128, 512], f32, tag="bvrep")
            with tc.tile_pool(name="ps_init", bufs=2, space="PSUM") as ps_init:
                # HAM warm-up: ~4.3us of dummy matmuls (no DMA deps) so the
                # PE clock gate releases (1.2 -> 2.4 GHz) before real work.
                trash = ps_init.tile([128, 512], f32, tag="trash", name="trash")
                NWARM = 16
                for dnum in range(NWARM):
                    pe_mm(
                        trash[:], ones_sb[:], warm_rhs[:],
                        start=(dnum == 0), stop=(dnum == NWARM - 1),
                    )
                end_blk()
                for p in range(NPAIR):
                    psb = ps_init.tile([128, 512], f32, tag="psb", name="psb")
                    pe_mm(
                        psb[:, 0:128], ones_sb[:], bv_sb[:, 128 * p : 128 * p + 128],
                        start=True, stop=True,
                    )
                    end_blk()
                    nc.vector.tensor_copy(
                        bvrep[:, 128 * p : 128 * p + 128], psb[:, 0:128]
                    )

            with tc.tile_pool(name="phBC", bufs=1) as pb:
              YT = pb.tile([128, NPAIR, T], bf16, tag="YT")
              with (
                  tc.tile_pool(name="qkt", bufs=2) as pqkt,
                  tc.tile_pool(name="phB_es", bufs=8) as pes,
                  tc.tile_pool(name="phB_rep", bufs=2) as prep,
                  tc.tile_pool(name="phC", bufs=3) as pc,
                  tc.tile_pool(name="ps_work", bufs=3, space="PSUM") as pwork,
                  tc.tile_pool(name="ps_Y", bufs=1, space="PSUM") as psy,
              ):
                qt_of = {}
                kt_of = {}

                def alloc_qkt(p):
                    qt_of[p] = pqkt.tile([128, T], bf16, tag="QTp", name="QTp")
                    kt_of[p] = pqkt.tile([128, T], bf16, tag="KTp", name="KTp")

                def work_tile():
                    return pwork.tile([128, 2, 512], f32, tag="pw", name="pw")

                def qk_unit(p, proj, tp):
                    """Q or K projection of pair p for t-chunk-pair tp
                    (columns 1024*tp .. 1024*tp+1024), K=1024 contraction in
                    one PSUM accumulation group per 512-wide half."""
                    def go():
                        if p not in qt_of:
                            alloc_qkt(p)
                        w_sb = wq_sb if proj == "q" else wk_sb
                        dest = qt_of[p] if proj == "q" else kt_of[p]
                        bias_sb = bq_sb if proj == "q" else bk_sb
                        ps = work_tile()
                        for half in range(2):
                            tch = 2 * tp + half
                            for kk in range(8):
                                pe_mm(
                                    ps[:, half, :],
                                    w_sb[:, kk, 128 * p : 128 * p + 128],
                                    xall[:, kk, 512 * tch : 512 * tch + 512],
                                    start=(kk == 0),
                                    stop=(kk == 7),
                                )
                        end_blk()
                        nc.vector.tensor_scalar_add(
                            dest[:, 1024 * tp : 1024 * tp + 1024],
                            ps.rearrange("s a b -> s (a b)"),
                            bias_sb[:, p : p + 1],
                        )
                    return go

                def v_unit(u):
                    """V for st-pair (2u, 2u+1), all pairs, K=1024 single
                    accumulation group per 512-wide half."""
                    def go():
                        ps = work_tile()
                        for half in range(2):
                            st = 2 * u + half
                            for kk in range(8):
                                pe_mm(
                                    ps[:, half, :],
                                    xall[:, kk, 128 * st : 128 * st + 128],
                                    wv_sb[:, kk, :],
                                    start=(kk == 0),
                                    stop=(kk == 7),
                                )
                        end_blk()
                        for half in range(2):
                            st = 2 * u + half
                            src4 = ps[:, half, :].rearrange(
                                "s (p two d) -> s p two d", p=4, two=2
                            )
                            b4 = bvrep.rearrange(
                                "s (p two d) -> s p two d", p=4, two=2
                            )
                            for hh in range(2):
                                nc.vector.tensor_add(
                                    V[:, :, st, hh, 0:64],
                                    src4[:, :, hh, :],
                                    b4[:, :, hh, :],
                                )
                    return go

                def c_unit(m):
                    """c_proj for t-block m: both 512-wide e-halves, summed
                    over pairs in PSUM, one bf16 copy + one DMA out."""
                    def go():
                        ps = work_tile()
                        for e in range(2):
                            for pp_ in range(NPAIR):
                                pe_mm(
                                    ps[:, e, :],
                                    YT[:, pp_, 128 * m : 128 * m + 128],
                                    wp_sb[:, pp_, 512 * e : 512 * e + 512],
                                    start=(pp_ == 0),
                                    stop=(pp_ == 3),
                                )
                        end_blk()
                        ob = pc.tile([128, 1024], bf16, tag="ob", name="ob")
                        nc.vector.tensor_copy(
                            ob[:], ps.rearrange("s a b -> s (a b)")
                        )
                        nc.sync.dma_start(
                            out[128 * m : 128 * m + 128, :], ob[:]
                        )
                    return go

                def emit_tail(p, j, psY):
                    """Two quick copies release the psY banks (~1.4us) so the
                    next chunk's AV isn't blocked (psy bufs=1); the rest of
                    the chain runs from SBUF. reciprocal_approx_fast only
                    works from SBUF at base partition 0."""
                    for hh in range(2):
                        ysb = prep.tile([64, 512], f32, tag="ysb", name="ysb")
                        nc.vector.tensor_copy(ysb[:], psY[hh][0:64, :])
                        row = prep.tile([1, 512], f32, tag="row", name="row")
                        nc.vector.tensor_copy(row[:], psY[hh][64:65, :])
                        rrow = prep.tile([1, 512], f32, tag="rrow", name="rrow")
                        nc.vector.reciprocal_approx_fast(
                            out=rrow[:], in_=row[:]
                        )
                        repc = prep.tile([64, 512], f32, tag="repc", name="repc")
                        nc.gpsimd.partition_broadcast(repc[:], rrow[:])
                        nc.vector.tensor_mul(
                            YT[64 * hh : 64 * hh + 64, p, 512 * j : 512 * j + 512],
                            ysb[:],
                            repc[:],
                        )

                # ---------------- preamble: QK0 t01, V st0-7 ----------------
                qk_unit(0, "q", 0)()
                qk_unit(0, "k", 0)()
                for u in range(4):
                    v_unit(u)()

                # ---------------- attention stages ----------------
                stage_fill = {
                    0: [qk_unit(0, "q", 1), qk_unit(0, "k", 1),
                        v_unit(4), v_unit(5), v_unit(6), v_unit(7),
                        qk_unit(1, "q", 0), qk_unit(1, "k", 0)],
                    1: [qk_unit(1, "q", 1), qk_unit(1, "k", 1),
                        qk_unit(2, "q", 0), qk_unit(2, "k", 0)],
                    2: [qk_unit(2, "q", 1), qk_unit(2, "k", 1),
                        qk_unit(3, "q", 0), qk_unit(3, "k", 0)],
                    3: [qk_unit(3, "q", 1), qk_unit(3, "k", 1)],
                }
                LOOK = 4  # S -> AV lookahead in (j, i) blocks

                for stage in range(NPAIR):
                    p = stage
                    filler = stage_fill[p]
                    fidx = 0
                    last_stage = stage == NPAIR - 1

                    blocks = []  # (j, i, last_of_chunk)
                    for j in range(NCH):
                        nst_j = 4 * j + 4
                        for i in range(nst_j):
                            blocks.append((j, i, i == nst_j - 1))
                    nblk = len(blocks)
                    nfe = len(filler) + (16 if last_stage else 0)

                    eS_store = {}
                    psY_of = {}
                    tails_pending = []

                    def pop_tails(n, filler=filler, last_stage=last_stage):
                        while tails_pending and tails_pending[0][0] <= n:
                            _, tp_, tj, tpsY = tails_pending.pop(0)
                            emit_tail(tp_, tj, tpsY)
                            if last_stage:
                                for m in range(4 * tj, 4 * tj + 4):
                                    filler.append(c_unit(m))

                    for n in range(nblk + LOOK):
                        pop_tails(n)
                        # AV block n-LOOK
                        if n >= LOOK:
                            j, i, last = blocks[n - LOOK]
                            psY = psY_of[j]
                            nst_j = 4 * j + 4
                            off = max(0, 128 * i - 512 * j)
                            eS = eS_store.pop((j, i))
                            for hh in range(2):
                                pe_mm(
                                    psY[hh][:, off:512],
                                    V[:, p, i, hh, 0:65],
                                    eS[:, hh, off:512],
                                    start=(i == 0),
                                    stop=(i == nst_j - 1),
                                )
                            end_blk()
                            if last:
                                tails_pending.append((n + 1, p, j, psY))
                        # filler unit(s), front-loaded, capped per block
                        want = min(
                            len(filler),
                            ((n + 1) * nfe) // max(1, int(0.6 * nblk)),
                            fidx + 2,
                        )
                        while fidx < want:
                            filler[fidx]()
                            fidx += 1
                        # S block n: both head-halves -> one wide exp
                        if n < nblk:
                            j, i, last = blocks[n]
                            if j not in psY_of:
                                psY_of[j] = [
                                    psy.tile(
                                        [65, 512], f32,
                                        tag=f"psY{hh}", name=f"psY{hh}",
                                    )
                                    for hh in range(2)
                                ]
                            off = max(0, 128 * i - 512 * j)
                            psS = work_tile()
                            for hh in range(2):
                                h0 = 64 * hh
                                pe_mm(
                                    psS[:, hh, off:512],
                                    kt_of[p][h0 : h0 + 64, 128 * i : 128 * i + 128],
                                    qt_of[p][
                                        h0 : h0 + 64,
                                        512 * j + off : 512 * j + 512,
                                    ],
                                    start=True,
                                    stop=True,
                                )
                            end_blk()
                            eS = pes.tile([128, 2, 512], bf16, tag="eS", name="eS")
                            nc.scalar.activation(
                                eS[:, :, off:512], psS[:, :, off:512], EXP,
                                scale=0.125,
                            )
                            if i >= 4 * j:
                                for hh in range(2):
                                    nc.vector.tensor_mul(
                                        eS[:, hh, off : off + 128],
                                        eS[:, hh, off : off + 128],
                                        tri_sb[:],
                                    )
                            eS_store[(j, i)] = eS
                    # drain: tails first (they append C units on stage 3)
                    pop_tails(10**9)
                    while fidx < len(filler):
                        filler[fidx]()
                        fidx += 1

    nc.compile()
    return nc


def _prep_core_inputs(x, Wq, bq, Wk, bk, Wv, bv, core):
    b, hg = core // 2, core % 2
    h0 = 8 * hg
    bft = ml_dtypes.bfloat16

    def wprep(W):
        A = W[h0 : h0 + 8]
        Bm = np.transpose(A, (2, 0, 1)).reshape(C, 512)
        return np.ascontiguousarray(
            Bm.reshape(8, 128, 512).transpose(1, 0, 2)
        ).astype(bft)

    def bprep(bias):
        return np.ascontiguousarray(bias[h0 : h0 + 8].reshape(4, 128).T)

    xT = x[b].T  # [C, T]
    xall = np.ascontiguousarray(
        xT.reshape(8, 128, T).transpose(1, 0, 2)
    ).astype(bft)

    return {
        "xall": xall,
        "wq": wprep(Wq),
        "wk": wprep(Wk),
        "wv": wprep(Wv),
        "bq": bprep(bq),
        "bk": bprep(bk),
        "bv": np.ascontiguousarray(bv[h0 : h0 + 8].reshape(1, 512)).astype(bft),
        "trimask": np.triu(np.ones((128, 128), np.float32)).astype(bft),
    }


def _prep_wp(Wp, hg):
    wp_sl = Wp[:, 512 * hg : 512 * hg + 512]
    return np.ascontiguousarray(
        wp_sl.T.reshape(4, 128, 1024).transpose(1, 0, 2)
    ).astype(ml_dtypes.bfloat16)


TRACE = False
TRACE_KW = {}


def kernel(x, Wq, bq, Wk, bk, Wv, bv, Wp, bp):
    x = np.asarray(x, np.float32)
    Wq = np.asarray(Wq, np.float32)
    bq = np.asarray(bq, np.float32)
    Wk = np.asarray(Wk, np.float32)
    bk = np.asarray(bk, np.float32)
    Wv = np.asarray(Wv, np.float32)
    bv = np.asarray(bv, np.float32)
    Wp = np.asarray(Wp, np.float32)
    bp = np.asarray(bp, np.float32)

    if "nc" not in _CACHE:
        _CACHE["nc"] = _build()
    nc = _CACHE["nc"]

    wp_of_hg = [_prep_wp(Wp, hg) for hg in range(2)]
    in_maps = []
    for core in range(NCORE):
        m = _prep_core_inputs(x, Wq, bq, Wk, bk, Wv, bv, core)
        m["wp"] = wp_of_hg[core % 2]
        in_maps.append(m)
    res = bass_utils.run_bass_kernel_spmd(
        nc, in_maps, list(range(NCORE)), trace=TRACE, **TRACE_KW
    )
    _CACHE["last_result"] = res

    outp = np.empty((B, T, C), np.float32)
    for b in range(B):
        outp[b] = (
            res.results[2 * b]["out"].astype(np.float32)
            + res.results[2 * b + 1]["out"].astype(np.float32)
            + bp
        )
    return outp


# revision 16
# speedup vs baseline: 1.1235x; 1.0483x over previous
"""Causal self-attention (separate heads) TRN2 Bass kernel — v3.

Problem (hardcoded): B=4, T=2048, C=1024, H=16, HS=64, fp32 in/out.
  q/k/v = per-head linear projections of x; att = softmax(causal(q k^T / 8));
  y = att v; out = concat_heads(y) @ Wp.T + bp.

Sharding over 8 NeuronCores: core c -> batch b = c//2, head-group hg = c%2
(8 heads each). Each core computes a [T, C] partial of the output; the host
sums the two partials per batch and adds bp.

All matmul operands bf16 (PSUM accumulation fp32); out DMA'd bf16 and summed
in fp32 on the host. x and all weights are DMA'd once and SBUF-resident.

Schedule (single PE stream, block-granular ordering):
  warm-up dummy MMs (HAM clock release) + bvrep broadcast
  QK0 t01  ->  V st0-7  (K=1024 single accumulation groups)
  stage p=0..3: per (j, i): two S MMs (head-halves h0/h64 row groups) into a
    2-bank psS2 -> ONE wide exp (ACT overhead is 352 cycles/instr) -> causal
    tri-mask mul -> AV with ones-augmented V -> chunk tails copy psY to SBUF
    (frees PSUM), reciprocal_approx_fast (SBUF only - PSUM reads broken),
    gpsimd broadcast, normalize mul into YT.
    Fillers: s0: QK0 t23 + V st8-15 + QK1; s1: QK1? -> see stage_fill; C units
    (c_proj per m-block, both e-halves, one DMA) interleave into stage 3 as
    its tails complete.
"""
import numpy as np
import ml_dtypes

from concourse import bacc, bass_utils, tile, mybir

B, T, C, H, HS = 4, 2048, 1024, 16, 64
NCORE = 8
NPAIR = 4
NCH = T // 512
NST = T // 128

f32 = mybir.dt.float32
bf16 = mybir.dt.bfloat16
EXP = mybir.ActivationFunctionType.Exp

_CACHE = {}


def _build():
    nc = bacc.Bacc(None, target_bir_lowering=False)

    xall_d = nc.declare_dram_parameter("xall", [128, 8, T], bf16, isOutput=False)
    wq_d = nc.declare_dram_parameter("wq", [128, 8, 512], bf16, isOutput=False)
    wk_d = nc.declare_dram_parameter("wk", [128, 8, 512], bf16, isOutput=False)
    wv_d = nc.declare_dram_parameter("wv", [128, 8, 512], bf16, isOutput=False)
    wp_d = nc.declare_dram_parameter("wp", [128, 4, 1024], bf16, isOutput=False)
    bq_d = nc.declare_dram_parameter("bq", [128, 4], f32, isOutput=False)
    bk_d = nc.declare_dram_parameter("bk", [128, 4], f32, isOutput=False)
    bv_d = nc.declare_dram_parameter("bv", [1, 512], bf16, isOutput=False)
    tri_d = nc.declare_dram_parameter("trimask", [128, 128], bf16, isOutput=False)
    out = nc.declare_dram_parameter("out", [T, C], bf16, isOutput=True)

    with tile.TileContext(nc) as tc:
        with tc.tile_pool(name="persist", bufs=1) as pp:
            # ---- constants / persistent tensors ----
            ones_sb = pp.tile([1, 128], bf16, tag="ones")
            warm_rhs = pp.tile([1, 512], bf16, tag="warmrow")
            onescol = pp.tile([128, 32], f32, tag="onescol")
            nc.vector.memset(ones_sb[:], 1.0)
            nc.vector.memset(warm_rhs[:], 1.0)
            nc.vector.memset(onescol[:], 1.0)

            tri_sb = pp.tile([128, 128], bf16, tag="tri")
            bq_sb = pp.tile([128, 4], f32, tag="bq")
            bk_sb = pp.tile([128, 4], f32, tag="bk")
            bv_sb = pp.tile([1, 512], bf16, tag="bv")
            xall = pp.tile([128, 8, T], bf16, tag="xall")
            wv_sb = pp.tile([128, 8, 512], bf16, tag="wv")
            wq_sb = pp.tile([128, 8, 512], bf16, tag="wq")
            wk_sb = pp.tile([128, 8, 512], bf16, tag="wk")
            wp_sb = pp.tile([128, 4, 1024], bf16, tag="wp")

            # DMA priority order: tiny consts, then what the preamble needs
            # first (xall + wq/wk for QK0, wv for V), then wp.
            nc.sync.dma_start(bv_sb[:], bv_d[:])
            nc.sync.dma_start(bq_sb[:], bq_d[:])
            nc.sync.dma_start(bk_sb[:], bk_d[:])
            nc.sync.dma_start(tri_sb[:], tri_d[:])
            nc.sync.dma_start(wq_sb[:], wq_d[:])
            nc.sync.dma_start(wk_sb[:], wk_d[:])
            for k in range(8):
                nc.sync.dma_start(xall[:, k, :], xall_d[:, k, :])
            nc.sync.dma_start(wv_sb[:], wv_d[:])
            nc.sync.dma_start(wp_sb[:], wp_d[:])

            # per-(pair, i, hh) block padded to 72 cols (144B) so every AV
            # lhsT slice is 16B-aligned (bf16 weight loads corrupt otherwise)
            V = pp.tile([128, NPAIR, NST, 2, 72], bf16, tag="V")
            for p in range(NPAIR):
                for i in range(NST):
                    nc.vector.tensor_copy(V[:, p, i, :, 64], onescol[:, 0:2])

            # PE emission-order chain, block granular
            _chain = {"prev": None, "first": None}

            def pe_mm(*args, **kw):
                inst = nc.tensor.matmul(*args, **kw)
                if _chain["first"] is None and _chain["prev"] is not None:
                    tile.add_dep_helper(
                        inst.ins, _chain["prev"].ins, sync=False,
                        reason="pe block order",
                    )
                if _chain["first"] is None:
                    _chain["first"] = inst
                _chain["prev"] = inst
                return inst

            def end_blk():
                _chain["first"] = None

            bvrep = pp.tile([128, 512], f32, tag="bvrep")
            with tc.tile_pool(name="ps_init", bufs=2, space="PSUM") as ps_init:
                # HAM warm-up: ~4.3us of dummy matmuls (no DMA deps) so the
                # PE clock gate releases (1.2 -> 2.4 GHz) before real work.
                trash = ps_init.tile([128, 512], f32, tag="trash", name="trash")
                NWARM = 16
                for dnum in range(NWARM):
                    pe_mm(
                        trash[:], ones_sb[:], warm_rhs[:],
                        start=(dnum == 0), stop=(dnum == NWARM - 1),
                    )
                end_blk()
                for p in range(NPAIR):
                    psb = ps_init.tile([128, 512], f32, tag="psb", name="psb")
                    pe_mm(
                        psb[:, 0:128], ones_sb[:], bv_sb[:, 128 * p : 128 * p + 128],
                        start=True, stop=True,
                    )
                    end_blk()
                    nc.vector.tensor_copy(
                        bvrep[:, 128 * p : 128 * p + 128], psb[:, 0:128]
                    )

            with tc.tile_pool(name="phBC", bufs=1) as pb:
              YT = pb.tile([128, NPAIR, T], bf16, tag="YT")
              with (
                  tc.tile_pool(name="qkt", bufs=2) as pqkt,
                  tc.tile_pool(name="phB_es", bufs=8) as pes,
                  tc.tile_pool(name="phB_rep", bufs=2) as prep,
                  tc.tile_pool(name="phC", bufs=3) as pc,
                  tc.tile_pool(name="ps_work", bufs=2, space="PSUM") as pwork,
                  tc.tile_pool(name="ps_Y", bufs=2, space="PSUM") as psy,
              ):
                qt_of = {}
                kt_of = {}

                def alloc_qkt(p):
                    qt_of[p] = pqkt.tile([128, T], bf16, tag="QTp", name="QTp")
                    kt_of[p] = pqkt.tile([128, T], bf16, tag="KTp", name="KTp")

                def work_tile():
                    return pwork.tile([128, 2, 512], f32, tag="pw", name="pw")

                def qk_unit(p, proj, tp):
                    """Q or K projection of pair p for t-chunk-pair tp
                    (columns 1024*tp .. 1024*tp+1024), K=1024 contraction in
                    one PSUM accumulation group per 512-wide half."""
                    def go():
                        if p not in qt_of:
                            alloc_qkt(p)
                        w_sb = wq_sb if proj == "q" else wk_sb
                        dest = qt_of[p] if proj == "q" else kt_of[p]
                        bias_sb = bq_sb if proj == "q" else bk_sb
                        ps = work_tile()
                        for half in range(2):
                            tch = 2 * tp + half
                            for kk in range(8):
                                pe_mm(
                                    ps[:, half, :],
                                    w_sb[:, kk, 128 * p : 128 * p + 128],
                                    xall[:, kk, 512 * tch : 512 * tch + 512],
                                    start=(kk == 0),
                                    stop=(kk == 7),
                                )
                        end_blk()
                        nc.vector.tensor_scalar_add(
                            dest[:, 1024 * tp : 1024 * tp + 1024],
                            ps.rearrange("s a b -> s (a b)"),
                            bias_sb[:, p : p + 1],
                        )
                    return go

                def v_unit(u):
                    """V for st-pair (2u, 2u+1), all pairs, K=1024 single
                    accumulation group per 512-wide half."""
                    def go():
                        ps = work_tile()
                        for half in range(2):
                            st = 2 * u + half
                            for kk in range(8):
                                pe_mm(
                                    ps[:, half, :],
                                    xall[:, kk, 128 * st : 128 * st + 128],
                                    wv_sb[:, kk, :],
                                    start=(kk == 0),
                                    stop=(kk == 7),
                                )
                        end_blk()
                        for half in range(2):
                            st = 2 * u + half
                            src4 = ps[:, half, :].rearrange(
                                "s (p two d) -> s p two d", p=4, two=2
                            )
                            b4 = bvrep.rearrange(
                                "s (p two d) -> s p two d", p=4, two=2
                            )
                            for hh in range(2):
                                nc.vector.tensor_add(
                                    V[:, :, st, hh, 0:64],
                                    src4[:, :, hh, :],
                                    b4[:, :, hh, :],
                                )
                    return go

                def c_unit(m):
                    """c_proj for t-block m: both 512-wide e-halves, summed
                    over pairs in PSUM, one bf16 copy + one DMA out."""
                    def go():
                        ps = work_tile()
                        for e in range(2):
                            for pp_ in range(NPAIR):
                                pe_mm(
                                    ps[:, e, :],
                                    YT[:, pp_, 128 * m : 128 * m + 128],
                                    wp_sb[:, pp_, 512 * e : 512 * e + 512],
                                    start=(pp_ == 0),
                                    stop=(pp_ == 3),
                                )
                        end_blk()
                        ob = pc.tile([128, 1024], bf16, tag="ob", name="ob")
                        nc.vector.tensor_copy(
                            ob[:], ps.rearrange("s a b -> s (a b)")
                        )
                        nc.sync.dma_start(
                            out[128 * m : 128 * m + 128, :], ob[:]
                        )
                    return go

                def emit_tail(p, j, psY):
                    """Two quick copies release the psY banks (~1.4us) so the
                    next chunk's AV isn't blocked (psy bufs=1); the rest of
                    the chain runs from SBUF. reciprocal_approx_fast only
                    works from SBUF at base partition 0."""
                    for hh in range(2):
                        row = prep.tile([1, 512], f32, tag="row", name="row")
                        nc.vector.tensor_copy(row[:], psY[hh][64:65, :])
                        rrow = prep.tile([1, 512], f32, tag="rrow", name="rrow")
                        nc.vector.reciprocal_approx_fast(
                            out=rrow[:], in_=row[:]
                        )
                        repc = prep.tile([64, 512], f32, tag="repc", name="repc")
                        nc.gpsimd.partition_broadcast(repc[:], rrow[:])
                        nc.vector.tensor_mul(
                            YT[64 * hh : 64 * hh + 64, p, 512 * j : 512 * j + 512],
                            psY[hh][0:64, :],
                            repc[:],
                        )

                # ---------------- preamble: QK0 t01, V st0-7 ----------------
                qk_unit(0, "q", 0)()
                qk_unit(0, "k", 0)()
                for u in range(4):
                    v_unit(u)()

                # ---------------- attention stages ----------------
                stage_fill = {
                    0: [qk_unit(0, "q", 1), qk_unit(0, "k", 1),
                        v_unit(4), v_unit(5), v_unit(6), v_unit(7),
                        qk_unit(1, "q", 0), qk_unit(1, "k", 0)],
                    1: [qk_unit(1, "q", 1), qk_unit(1, "k", 1),
                        qk_unit(2, "q", 0), qk_unit(2, "k", 0)],
                    2: [qk_unit(2, "q", 1), qk_unit(2, "k", 1),
                        qk_unit(3, "q", 0), qk_unit(3, "k", 0)],
                    3: [qk_unit(3, "q", 1), qk_unit(3, "k", 1)],
                }
                LOOK = 4  # S -> AV lookahead in (j, i) blocks

                for stage in range(NPAIR):
                    p = stage
                    filler = stage_fill[p]
                    fidx = 0
                    last_stage = stage == NPAIR - 1

                    blocks = []  # (j, i, last_of_chunk)
                    for j in range(NCH):
                        nst_j = 4 * j + 4
                        for i in range(nst_j):
                            blocks.append((j, i, i == nst_j - 1))
                    nblk = len(blocks)
                    nfe = len(filler) + (16 if last_stage else 0)

                    eS_store = {}
                    psY_of = {}
                    tails_pending = []

                    def pop_tails(n, filler=filler, last_stage=last_stage):
                        while tails_pending and tails_pending[0][0] <= n:
                            _, tp_, tj, tpsY = tails_pending.pop(0)
                            emit_tail(tp_, tj, tpsY)
                            if last_stage:
                                for m in range(4 * tj, 4 * tj + 4):
                                    filler.append(c_unit(m))

                    for n in range(nblk + LOOK):
                        pop_tails(n)
                        # AV block n-LOOK
                        if n >= LOOK:
                            j, i, last = blocks[n - LOOK]
                            psY = psY_of[j]
                            nst_j = 4 * j + 4
                            off = max(0, 128 * i - 512 * j)
                            eS = eS_store.pop((j, i))
                            for hh in range(2):
                                pe_mm(
                                    psY[hh][:, off:512],
                                    V[:, p, i, hh, 0:65],
                                    eS[:, hh, off:512],
                                    start=(i == 0),
                                    stop=(i == nst_j - 1),
                                )
                            end_blk()
                            if last:
                                tails_pending.append((n + 1, p, j, psY))
                        # filler unit(s), front-loaded, capped per block
                        want = min(
                            len(filler),
                            ((n + 1) * nfe) // max(1, int(0.45 * nblk)),
                            fidx + 2,
                        )
                        while fidx < want:
                            filler[fidx]()
                            fidx += 1
                        # S block n: both head-halves -> one wide exp
                        if n < nblk:
                            j, i, last = blocks[n]
                            if j not in psY_of:
                                psY_of[j] = [
                                    psy.tile(
                                        [65, 512], f32,
                                        tag=f"psY{hh}", name=f"psY{hh}",
                                    )
                                    for hh in range(2)
                                ]
                            off = max(0, 128 * i - 512 * j)
                            psS = work_tile()
                            for hh in range(2):
                                h0 = 64 * hh
                                pe_mm(
                                    psS[:, hh, off:512],
                                    kt_of[p][h0 : h0 + 64, 128 * i : 128 * i + 128],
                                    qt_of[p][
                                        h0 : h0 + 64,
                                        512 * j + off : 512 * j + 512,
                                    ],
                                    start=True,
                                    stop=True,
                                )
                            end_blk()
                            eS = pes.tile([128, 2, 512], bf16, tag="eS", name="eS")
                            nc.scalar.activation(
                                eS[:, :, off:512], psS[:, :, off:512], EXP,
                                scale=0.125,
                            )
                            if i >= 4 * j:
                                for hh in range(2):
                                    nc.vector.tensor_mul(
                                        eS[:, hh, off : off + 128],
                                        eS[:, hh, off : off + 128],
                                        tri_sb[:],
                                    )
                            eS_store[(j, i)] = eS
                    # drain: tails first (they append C units on stage 3)
                    pop_tails(10**9)
                    while fidx < len(filler):
                        filler[fidx]()
                        fidx += 1

    nc.compile()
    return nc


def _prep_core_inputs(x, Wq, bq, Wk, bk, Wv, bv, core):
    b, hg = core // 2, core % 2
    h0 = 8 * hg
    bft = ml_dtypes.bfloat16

    def wprep(W):
        A = W[h0 : h0 + 8]
        Bm = np.transpose(A, (2, 0, 1)).reshape(C, 512)
        return np.ascontiguousarray(
            Bm.reshape(8, 128, 512).transpose(1, 0, 2)
        ).astype(bft)

    def bprep(bias):
        return np.ascontiguousarray(bias[h0 : h0 + 8].reshape(4, 128).T)

    xT = x[b].T  # [C, T]
    xall = np.ascontiguousarray(
        xT.reshape(8, 128, T).transpose(1, 0, 2)
    ).astype(bft)

    return {
        "xall": xall,
        "wq": wprep(Wq),
        "wk": wprep(Wk),
        "wv": wprep(Wv),
        "bq": bprep(bq),
        "bk": bprep(bk),
        "bv": np.ascontiguousarray(bv[h0 : h0 + 8].reshape(1, 512)).astype(bft),
        "trimask": np.triu(np.ones((128, 128), np.float32)).astype(bft),
    }


def _prep_wp(Wp, hg):
    wp_sl = Wp[:, 512 * hg : 512 * hg + 512]
    return np.ascontiguousarray(
        wp_sl.T.reshape(4, 128, 1024).transpose(1, 0, 2)
    ).astype(ml_dtypes.bfloat16)


TRACE = False
TRACE_KW = {}


def kernel(x, Wq, bq, Wk, bk, Wv, bv, Wp, bp):
    x = np.asarray(x, np.float32)
    Wq = np.asarray(Wq, np.float32)
    bq = np.asarray(bq, np.float32)
    Wk = np.asarray(Wk, np.float32)
    bk = np.asarray(bk, np.float32)
    Wv = np.asarray(Wv, np.float32)
    bv = np.asarray(bv, np.float32)
    Wp = np.asarray(Wp, np.float32)
    bp = np.asarray(bp, np.float32)

    if "nc" not in _CACHE:
        _CACHE["nc"] = _build()
    nc = _CACHE["nc"]

    wp_of_hg = [_prep_wp(Wp, hg) for hg in range(2)]
    in_maps = []
    for core in range(NCORE):
        m = _prep_core_inputs(x, Wq, bq, Wk, bk, Wv, bv, core)
        m["wp"] = wp_of_hg[core % 2]
        in_maps.append(m)
    res = bass_utils.run_bass_kernel_spmd(
        nc, in_maps, list(range(NCORE)), trace=TRACE, **TRACE_KW
    )
    _CACHE["last_result"] = res

    outp = np.empty((B, T, C), np.float32)
    for b in range(B):
        outp[b] = (
            res.results[2 * b]["out"].astype(np.float32)
            + res.results[2 * b + 1]["out"].astype(np.float32)
            + bp
        )
    return outp
